# revision 3
# baseline (speedup 1.0000x reference)
# Trainium2 Bass kernel for nn_Generator (2-layer LSTM music generator).
# Data-parallel across 8 NeuronCores: 8 samples/core, weights replicated.
#
# v5: v3 + split-fp16 residual h: the PE consumes h as
# h_hi = fp16(h) plus h_res = fp16(h - h_hi), with each K-chunk round
# run twice (hi, res) accumulating into the same PSUM rows. This
# restores ~21-bit effective h precision (C err ~1e-6, f32r-grade) while
# keeping 4-way col-tiled 16-bit streams. Col-tile j computes
# gate-slice j: its 512 weight columns are the host-permuted
# [i_j | f_j | o_j | g_j] (128 each, x_j = dims 128j..128j+128 of gate x).
# All four gate types for sample b of slice j land on PSUM partition
# 32j+b, so the LSTM elementwise runs on [104, 128] tiles in fp32
# (c and gates stay fp32; only matmul inputs are bf16). h lives as
# [104, 128] (partition = 32*slice + sample, free = within-slice dim);
# one PE transpose per layer rebuilds the [128, 128] bf16 stationary hT.
import numpy as np

B, T, L = 64, 512, 1024
ND, EMB = 512, 32
H4 = 4 * ND
NCORES = 8
BS = 8
HEAD_SIZES = (24, 12, 6, 4, 2, 10, 10, 2, 10, 10, 3, 10, 10, 10)
OFFSETS = (2, 0, 0, 0, 26, 0, 0, 21, 0, 0, 23, 0, 0, 0)
END_TOK = 1

_CACHE = {}


def _build(nt):
    import concourse.bacc as bacc
    import concourse.tile as tile
    from concourse import mybir

    dt = mybir.dt
    F32 = dt.float32
    F16 = dt.float16
    AF = mybir.ActivationFunctionType

    nc = bacc.Bacc(trn_type="TRN2")

    # ---- DRAM I/O (per core) ----
    d_emb = nc.dram_tensor("embt", [33, nt * BS], F16, kind="ExternalInput")
    d_w1 = nc.dram_tensor("w1", [128, 4 * 4 * 512], F16, kind="ExternalInput")
    d_w2 = nc.dram_tensor("w2", [128, 8 * 4 * 512], F16, kind="ExternalInput")
    d_wi1 = nc.dram_tensor("wi1", [33, 4 * 512], F16, kind="ExternalInput")
    d_b2 = nc.dram_tensor("b2p", [65, 4 * 512], F16, kind="ExternalInput")
    d_id = nc.dram_tensor("idt", [104, 104], F32, kind="ExternalInput")
    d_wl = nc.dram_tensor("w_lstm", [1, L], F32, kind="ExternalInput")
    d_lstm = nc.dram_tensor("lstm", [BS, L, L], F32, kind="ExternalInput")

    d_ct = nc.dram_tensor("out_ct", [nt, 104, 128], F32, kind="ExternalOutput")
    d_out_ptr = nc.dram_tensor("out_ptr", [128, BS, L // 128], F32,
                               kind="ExternalOutput")

    with tile.TileContext(nc) as tc:
        with (
            tc.tile_pool(name="wt", bufs=1) as wt,
            tc.tile_pool(name="state", bufs=2) as st,
            tc.tile_pool(name="sgp", bufs=2) as sgp,
            tc.tile_pool(name="tmps", bufs=4) as tp_,
            tc.tile_pool(name="hout", bufs=3) as ho,
            tc.tile_pool(name="lst", bufs=2) as lp,
            tc.tile_pool(name="gps", bufs=1, space="PSUM") as gpsum,
            tc.tile_pool(name="hps", bufs=2, space="PSUM") as hpsum,
        ):
            # ---- load weights/constants ----
            embT = wt.tile([33, nt, BS], F16)
            nc.sync.dma_start(out=embT, in_=d_emb.rearrange(
                "p (t b) -> p t b", b=BS))
            w1 = wt.tile([128, 4, 4, 512], F16)
            nc.sync.dma_start(out=w1, in_=d_w1.rearrange(
                "p (r j n) -> p r j n", r=4, j=4))
            w2 = wt.tile([128, 8, 4, 512], F16)
            nc.sync.dma_start(out=w2, in_=d_w2.rearrange(
                "p (r j n) -> p r j n", r=8, j=4))
            wi1 = wt.tile([33, 4, 512], F16)
            nc.sync.dma_start(out=wi1, in_=d_wi1.rearrange(
                "p (j n) -> p j n", j=4))
            # b2 rhs lives on partition 64 (row-group 2) so the b2 inject
            # can row-tile concurrently with the emb inject (rows 0:64)
            b2p = wt.tile([65, 4, 512], F16)
            nc.sync.dma_start(out=b2p, in_=d_b2.rearrange(
                "p (j n) -> p j n", j=4))
            idt = wt.tile([104, 104], F32)
            nc.sync.dma_start(out=idt, in_=d_id[:, :])
            wl = wt.tile([128, L], F32)
            nc.sync.dma_start(out=wl, in_=d_wl.broadcast_to([128, L]))
            ones1 = wt.tile([65, 32], F16)
            nc.vector.memset(ones1, 1.0)
            ptr_sb = wt.tile([128, BS, L // 128], F32)

            # states: h/c as [104, 128] (partition 32j+b, free d)
            c1 = st.tile([104, 128], F32, tag="c1")
            c2 = st.tile([104, 128], F32, tag="c2")
            h1Thi = st.tile([128, 128], F16, tag="h1Thi")
            h1Tres = st.tile([128, 128], F16, tag="h1Tres")
            h2Thi = st.tile([128, 128], F16, tag="h2Thi")
            h2Tres = st.tile([128, 128], F16, tag="h2Tres")
            nc.vector.memset(c1, 0.0)
            nc.vector.memset(c2, 0.0)
            for hh in (h1Thi, h1Tres, h2Thi, h2Tres):
                nc.vector.memset(hh, 0.0)
            h1T = (h1Thi, h1Tres)
            h2T = (h2Thi, h2Tres)
            emb_st = st.tile([33, 2, 32], F16, tag="embst")
            nc.vector.memset(emb_st, 0.0)

            nptr = BS * (L // 128)
            ptr_done = 0

            def ptr_tile(k):
                b, j = divmod(k, L // 128)
                lt = lp.tile([128, L], F32, tag="lt")
                nc.sync.dma_start(out=lt, in_=d_lstm[b, 128 * j:128 * (j + 1), :])
                prod = lp.tile([128, L], F32, tag="prod")
                nc.vector.scalar_tensor_tensor(
                    out=prod, in0=lt, scalar=1.0, in1=wl,
                    op0=mybir.AluOpType.bypass, op1=mybir.AluOpType.mult,
                    accum_out=ptr_sb[:, b, j:j + 1])

            def gate_rounds(g, hT, w, woff, coff, rn, start,
                            stop_last=False):
                # hT = (hi, res) fp16 pair; chunk coff+r contracts weight
                # k-block woff+r. All hi rounds stream first so the PE is
                # not gated on the residual (ready ~600 ns after hi).
                hi, res = hT
                for half, ht in ((0, hi), (1, res)):
                    for r in range(rn):
                        for j in range(4):
                            nc.tensor.matmul(
                                g[32 * j:32 * (j + 1), :],
                                ht[:, 32 * (coff + r):32 * (coff + r + 1)],
                                w[:, woff + r, j, :],
                                start=(start and r == 0 and half == 0),
                                stop=(stop_last and r == rn - 1 and half == 1),
                                tile_position=(0, 32 * j))

            def lstm_post(layer, t, g, c_in, hT_new):
                # g: [128, 512] psum, cols [i|f|o|g] slices of 128
                sg = sgp.tile([104, 512], F32, tag=f"sg{layer}")
                nc.scalar.activation(sg[:, 0:384], g[0:104, 0:384], AF.Sigmoid)
                nc.scalar.activation(sg[:, 384:512], g[0:104, 384:512], AF.Tanh)
                u = tp_.tile([104, 128], F32, tag="u")
                nc.vector.tensor_mul(u, sg[:, 0:128], sg[:, 384:512])
                v = tp_.tile([104, 128], F32, tag="v")
                nc.vector.tensor_mul(v, sg[:, 128:256], c_in)
                c_new = st.tile([104, 128], F32, tag=f"c{layer}")
                nc.vector.tensor_add(c_new, u, v)
                tc_ = tp_.tile([104, 128], F32, tag="tc")
                nc.scalar.activation(tc_, c_new, AF.Tanh)
                h = ho.tile([104, 128], F32, tag=f"h{layer}")
                nc.vector.tensor_mul(h, sg[:, 256:384], tc_)
                # one transpose: [104, 128] -> [128, 104] psum, then split
                # into fp16 hi + fp16 residual
                hp = hpsum.tile([128, 104], F32, tag="hp")
                nc.tensor.transpose(hp, h, idt)
                hi, res = hT_new
                nc.vector.tensor_copy(hi[:, 0:104], hp)
                hi32 = tp_.tile([128, 104], F32, tag="hi32")
                nc.vector.tensor_copy(hi32, hi[:, 0:104])
                nc.vector.scalar_tensor_tensor(
                    out=res[:, 0:104], in0=hp, scalar=1.0, in1=hi32,
                    op0=mybir.AluOpType.bypass, op1=mybir.AluOpType.subtract)
                return c_new, h

            for t in range(nt):
                e = t % 2
                # L1: Wh1 rounds (critical chain first)
                g1 = gpsum.tile([128, 512], F32, tag=f"g1{e}")
                gate_rounds(g1, h1T, w1, 0, 0, 4, True)
                # L2 part A chunk 0 opens g2's group so injects can follow
                g2 = gpsum.tile([128, 512], F32, tag=f"g2{e}")
                gate_rounds(g2, h2T, w2, 4, 0, 1, True)
                # emb inject (rows 0:64) + b2 inject (row 64): row+col tiled
                nc.vector.tensor_copy(emb_st[:, e, 0:8], embT[:, t, :])
                for j in range(4):
                    nc.tensor.matmul(
                        g1[32 * j:32 * (j + 1), :],
                        emb_st[:, e, :], wi1[:, j, :],
                        start=False, stop=True, tile_position=(0, 32 * j))
                for j in range(4):
                    nc.tensor.matmul(
                        g2[32 * j:32 * (j + 1), :],
                        ones1[64:65, :], b2p[64:65, j, :],
                        start=False, stop=False, tile_position=(64, 32 * j))
                # L2 part A chunks 1-3 (fill PE while L1 elementwise runs)
                gate_rounds(g2, h2T, w2, 5, 1, 3, False)

                h1T_new = (st.tile([128, 128], F16, name="h1hi", tag="h1Thi"),
                           st.tile([128, 128], F16, name="h1re", tag="h1Tres"))
                c1, h1 = lstm_post(1, t, g1, c1, h1T_new)
                h1T = h1T_new

                # L2 part B: Wi2 (h1 chunks), stop on last MM
                gate_rounds(g2, h1T, w2, 0, 0, 4, False, stop_last=True)

                h2T_new = (st.tile([128, 128], F16, name="h2hi", tag="h2Thi"),
                           st.tile([128, 128], F16, name="h2re", tag="h2Tres"))
                c2, h2 = lstm_post(2, t, g2, c2, h2T_new)
                h2T = h2T_new
                nc.sync.dma_start(out=d_ct[t, :, :], in_=h2)

                while ptr_done * nt < (t + 1) * nptr:
                    ptr_tile(ptr_done)
                    ptr_done += 1

            while ptr_done < nptr:
                ptr_tile(ptr_done)
                ptr_done += 1

            nc.sync.dma_start(out=d_out_ptr[:, :, :], in_=ptr_sb)

    nc.finalize()
    return nc


def _get_nc(nt):
    if nt not in _CACHE:
        _CACHE[nt] = _build(nt)
    return _CACHE[nt]


def _perm_cols():
    # col c of tile j covers [i_j | f_j | o_j | g_j]; torch gate order in
    # H4 rows is (i, f, g, o)
    off = [0, 1, 3, 2]
    perm = np.empty((4, 512), np.int64)
    for j in range(4):
        for c in range(512):
            perm[j, c] = off[c // 128] * 512 + 128 * j + (c % 128)
    return perm


def _bf16(x):
    return np.asarray(x, np.float32).astype(np.float16)


def _host_pre(inputs, nt):
    f32 = np.float32
    trees = np.asarray(inputs["trees"])
    lstm_out = np.ascontiguousarray(np.asarray(inputs["lstm_out"], f32))
    et = np.asarray(inputs["embed_table"], f32)
    emb = et[trees[:, :, 2]]  # [B, T, EMB]

    perm = _perm_cols().reshape(-1)  # [2048]
    wh1 = np.asarray(inputs["Wh1"], f32)  # [H4, ND]
    wi2 = np.asarray(inputs["Wi2"], f32)
    wh2 = np.asarray(inputs["Wh2"], f32)
    wi1 = np.asarray(inputs["Wi1"], f32)  # [H4, EMB]
    b1 = np.asarray(inputs["b1"], f32)
    b2 = np.asarray(inputs["b2"], f32)

    def pack_h(wmat, nr):  # wmat [H4, 128*nr] -> [128, nr*4*512]
        w = wmat[perm, :]  # [2048, nr*128]
        w = w.reshape(4, 512, nr, 128).transpose(3, 2, 0, 1)
        return np.ascontiguousarray(w.reshape(128, nr * 4 * 512))

    w1 = pack_h(wh1, 4)
    w2 = pack_h(np.concatenate([wi2, wh2], axis=1), 8)
    wi1_aug = np.concatenate([wi1.T, b1[None, :]], axis=0)  # [33, H4]
    wi1p = np.ascontiguousarray(wi1_aug[:, perm].reshape(33, 4 * 512))
    b2p = np.zeros((65, 4 * 512), f32)
    b2p[64, :] = b2[perm]

    idt = np.eye(104, dtype=f32)
    shared = dict(
        w1=_bf16(w1), w2=_bf16(w2), wi1=_bf16(wi1p), b2p=_bf16(b2p),
        idt=idt,
        w_lstm=np.ascontiguousarray(
            np.asarray(inputs["ptrW"], f32)[0, ND:][None, :]),
    )
    per_core = []
    for c in range(NCORES):
        sl = slice(c * BS, (c + 1) * BS)
        embT = np.transpose(emb[sl, :nt, :], (2, 1, 0))  # [EMB, nt, BS]
        embT_aug = np.concatenate([embT, np.ones((1, nt, BS), f32)], axis=0)
        m = dict(shared)
        m["embt"] = _bf16(embT_aug.reshape(33, nt * BS))
        m["lstm"] = lstm_out[sl]
        per_core.append(m)
    first = np.asarray(inputs["first_notes"], f32)
    return per_core, dict(trees=trees, first=first, et=et)


def _host_post(results, ctx, inputs, nt):
    f32 = np.float32
    trees, first, et = ctx["trees"], ctx["first"], ctx["et"]
    Bn = trees.shape[0]
    # out_ct [nt, 104, 128]: C[b, t, 128j + d] = out[t, 32j + b, d]
    Cs = []
    for r in results:
        oc = np.asarray(r["out_ct"], f32).reshape(nt, 104, 128)
        c = np.empty((BS, nt, ND), f32)
        for j in range(4):
            c[:, :, 128 * j:128 * (j + 1)] = np.transpose(
                oc[:, 32 * j:32 * j + 8, :], (1, 0, 2))
        Cs.append(c)
    C = np.concatenate(Cs, axis=0)
    lstm_dot = np.concatenate(
        [np.transpose(np.asarray(r["out_ptr"], f32), (1, 2, 0)).reshape(BS, L)
         for r in results], axis=0)

    nw = np.asarray(inputs["next_W"], f32)
    nb = np.asarray(inputs["next_b"], f32)
    base = first @ (nw[:, :ND] + nw[:, ND:2 * ND]).T + nb  # [B, ND]
    w3 = nw[:, 2 * ND:]
    final = base + C[:, -1, :] @ w3.T
    u = final @ w3
    S = np.empty((Bn, nt), f32)
    S[:, 0] = np.sum(first * final, axis=1)
    S[:, 1:] = (np.einsum("btd,bd->bt", C[:, :nt - 1, :], u)
                + np.sum(base * final, axis=1)[:, None])

    rows = np.arange(Bn)
    idx = trees[:, -1, 0].astype(np.int64) + 1
    is_end = trees[:, -1, 2] == END_TOK
    alt = np.clip(idx - trees[:, -1, 1] - 1, 0, nt - 1).astype(np.int64)
    parent_idx = np.where(is_end, trees[rows, alt, 1], trees[:, -1, 0])
    parent_type = trees[rows, np.clip(parent_idx, 0, nt - 1).astype(np.int64), 2]
    parent_embed = et[parent_type]

    top5 = np.argsort(-S, axis=1, kind="stable")[:, :5]
    top_types = trees[rows[:, None], top5, 2]
    reord = et[top_types].reshape(Bn, 5 * EMB)

    h = np.maximum(reord @ np.asarray(inputs["attW1"], f32).T
                   + np.asarray(inputs["attb1"], f32), 0)
    h = np.maximum(h @ np.asarray(inputs["attW2"], f32).T
                   + np.asarray(inputs["attb2"], f32), 0)
    temp = np.maximum(
        np.concatenate([parent_embed, h], axis=1)
        @ np.asarray(inputs["combW"], f32).T + np.asarray(inputs["combb"], f32), 0)
    logits = temp @ np.asarray(inputs["headsW"], f32).T + np.asarray(
        inputs["headsb"], f32)
    splits = np.cumsum(HEAD_SIZES)[:-1].tolist()
    picks = [np.argmax(p, axis=1) + off
             for p, off in zip(np.split(logits, splits, axis=1), OFFSETS)]
    ptrW = np.asarray(inputs["ptrW"], f32)
    ptr_logits = (temp @ ptrW[0, :ND])[:, None] + lstm_dot + np.asarray(
        inputs["ptrb"], f32)[0]
    ptr_pick = np.argmax(ptr_logits, axis=1)
    cols = [idx, parent_idx] + picks + [ptr_pick]
    return np.stack([np.asarray(c, np.int32) for c in cols], axis=1)


def kernel(**inputs):
    from concourse.bass_utils import run_bass_kernel_spmd
    nt = T
    per_core, ctx = _host_pre(inputs, nt)
    nc = _get_nc(nt)
    res = run_bass_kernel_spmd(nc, per_core, core_ids=list(range(NCORES)))
    return _host_post(res.results, ctx, inputs, nt)
